# revision 31
# baseline (speedup 1.0000x reference)
"""Ragged-segment attention for Trainium2 (8 NeuronCores, SPMD), bin-dense fp16.

Per-segment masking/softmax structure is folded into a host-built low-rank
additive mask applied with ONE matmul per bin:
    mask[q,k] = (kb[k] + NEG) * 1  +  sum_s (-NEG) * 1_s[q] 1_s[k]
so scores/softmax/exp-transpose/out are all dense [128 x 128] bin ops and
segments pack at arbitrary offsets (first-fit decreasing, ~94% dense bins).

The feature-major context copy (ctT) is pre-transposed on the HOST and DMA'd
directly, so the PE runs a pure matmul stream (no transpose->PSUM->copy
chains): per 4-bin group 16 u-matmuls, 4x(4 score + 1 mask) matmuls, 4 exp
transposes and 4 out matmuls = 13312 PE cycles.  The 3-deep software
pipeline (load g+2 / u-matmul g+1 / softmax+out g) keeps the in-order PE
from idling while Act/DVE run the softmax stats.

DMAs are batched per 4-bin group (context both layouts, masks, outputs)
because each DMA instruction costs ~625ns of serialized HWDGE
descriptor-generation time.
"""
import numpy as np

import concourse.bacc as bacc
import concourse.mybir as mybir
import concourse.tile as tile
from concourse.bass_utils import run_bass_kernel_spmd

F32 = mybir.dt.float32
FP16 = mybir.dt.float16

N_CORES = 8
D = 512
BIN = 128
GROUP = 4

DEFAULT_MODE = "fp16T"

LAST_RESULTS = {}


def _plan(lengths, mode=None):
    S = len(lengths)
    n_slots = S // N_CORES
    order = np.argsort(-lengths, kind="stable")
    seg_ids = [[int(order[N_CORES * j + c]) for j in range(n_slots)]
               for c in range(N_CORES)]
    slot_len = [int(lengths[order[N_CORES * j]]) for j in range(n_slots)]

    bins = []   # (used-token count, n_segs) per bin
    slots = []  # (bin, off, L)
    for j, L in enumerate(slot_len):
        bi = next((i for i, (used, ns) in enumerate(bins)
                   if used + L <= BIN and ns < 31), None)
        if bi is None:
            bins.append((0, 0))
            bi = len(bins) - 1
        used, ns = bins[bi]
        slots.append((bi, used, L))
        bins[bi] = (used + L, ns + 1)
    n_bins = ((len(bins) + GROUP - 1) // GROUP) * GROUP
    return slots, n_bins, seg_ids


def _mask_layout(slots, n_bins):
    by_bin = [[] for _ in range(n_bins)]
    for bi, off, L in slots:
        by_bin[bi].append((off, L))
    kmask = [len(by_bin[b]) + 1 for b in range(n_bins)]
    assert max(kmask) <= 32
    return by_bin, kmask


def _build(slots, n_bins, mode=None, repeat=1):
    DT = FP16
    T = n_bins * BIN
    n_groups = n_bins // GROUP
    nc = bacc.Bacc("TRN2", target_bir_lowering=False)

    by_bin, kmask = _mask_layout(slots, n_bins)

    cpk = nc.dram_tensor("cpk", [T, D], DT, kind="ExternalInput")
    # feature-major context, host-transposed: row g*128+p holds, for
    # d-partition p of group g, all [k-chunk][bin][token] values
    ctt = nc.dram_tensor("ctt", [n_groups * 128, 4 * GROUP * 128], DT,
                         kind="ExternalInput")
    wt = nc.dram_tensor("wt", [128, 4 * D], DT, kind="ExternalInput")
    bvec = nc.dram_tensor("bvec", [128, 4], F32, kind="ExternalInput")
    # per-group mask rows: bin i of a group at partitions [32i, 32i+km)
    msk = nc.dram_tensor("msk", [n_groups * 128, GROUP * 128], mybir.dt.float8e5,
                         kind="ExternalInput")
    opk = nc.dram_tensor("opk", [T, D], DT, kind="ExternalOutput")

    ident = nc.inline_tensor(np.eye(128, dtype=np.float16), name="ident")

    with tile.TileContext(nc) as tc:
        with (
            tc.tile_pool(name="const", bufs=1) as cpool,
            tc.tile_pool(name="cb", bufs=4) as cbp,
            tc.tile_pool(name="ctp", bufs=3) as ctp,
            tc.tile_pool(name="utp", bufs=2) as utp,
            tc.tile_pool(name="seg", bufs=10) as segp,
            tc.tile_pool(name="stat", bufs=18) as statp,
            tc.tile_pool(name="outp", bufs=3) as outp,
            tc.tile_pool(name="mk", bufs=3) as mkp,
            tc.tile_pool(name="ups", bufs=2, space="PSUM") as ups,
            tc.tile_pool(name="scps", bufs=2, space="PSUM") as scps,
            tc.tile_pool(name="teps", bufs=2, space="PSUM") as teps,
            tc.tile_pool(name="ops", bufs=2, space="PSUM") as opsp,
        ):
            wt_sb = cpool.tile([128, 4, D], DT, tag="wt")
            b_sb = cpool.tile([128, 4], F32, tag="b")
            id_t = cpool.tile([128, 128], DT, tag="id")
            mg_all = cpool.tile([128, n_groups, GROUP, 128], mybir.dt.float8e5,
                                tag="mg_all")
            cg_all = cpool.tile([128, n_groups, GROUP, D], DT, tag="cg_all")
            ct_all = cpool.tile([128, n_groups, 4, GROUP, 128], DT, tag="ct_all")
            nc.sync.dma_start(wt_sb[:], wt.ap().rearrange("p (c e) -> p c e", c=4))
            nc.sync.dma_start(b_sb[:], bvec[:])
            nc.sync.dma_start(id_t[:], ident[:])
            # all groups' mask rows stay SBUF-resident (4.25KB/partition):
            # saves a 32KB DMA per group of steady-state HBM traffic
            nc.sync.dma_start(
                mg_all[:], msk.ap().rearrange("(g r) (i p) -> r g i p",
                                              i=GROUP, g=n_groups))
            # the WHOLE packed context stays SBUF-resident in both layouts
            # (68KB/partition each): steady-state HBM traffic is then just
            # the output store
            cpk_r = cpk.ap().rearrange("(g i p) d -> p g i d", p=BIN, i=GROUP)
            ctt_r = ctt.ap().rearrange("(g p) (k i t) -> p g k i t",
                                       p=128, k=4, i=GROUP)
            for gq in range(0, n_groups, 5):
                ge = min(gq + 5, n_groups)
                nc.sync.dma_start(cg_all[:, gq:ge], cpk_r[:, gq:ge])
                nc.gpsimd.dma_start(ct_all[:, gq:ge], ctt_r[:, gq:ge])

            cpk_v = cpk.ap().rearrange("(b p) d -> p b d", p=BIN)
            opk_v = opk.ap().rearrange("(b p) d -> p b d", p=BIN)
            ctt_v = ctt.ap().rearrange("(g p) (k i t) -> g p k i t",
                                       p=128, k=4, i=GROUP)

            # non-empty bins are a prefix of each group (packing fills bins
            # in order), so per-group work/stores cover just the first nb
            nb_used = [sum(1 for i in range(GROUP) if by_bin[g * GROUP + i])
                       for g in range(n_groups)]

            def load_group(g):
                """Context is resident: just bind the group's slices."""
                return {"g": g, "nb": nb_used[g],
                        "cg": cg_all[:, g], "ct": ct_all[:, g]}

            def u_chunk(st, c):
                ct, nb = st["ct"], st["nb"]
                if c == 0:
                    ut_t = utp.tile([128, 4, GROUP, 128], DT, tag="ut")
                    st["ut"] = ut_t
                ups_t = ups.tile([128, GROUP * 128], F32, tag="ups")
                for k in range(4):
                    nc.tensor.matmul(
                        ups_t[:, :nb * 128], wt_sb[:, k, c * 128:(c + 1) * 128],
                        ct[:, k, :nb, :], start=(k == 0), stop=(k == 3))
                nc.scalar.activation(
                    st["ut"][:, c, :nb, :], ups_t[:, :nb * 128],
                    mybir.ActivationFunctionType.Tanh, bias=b_sb[:, c:c + 1])

            def bin_scores(st, i):
                g = st["g"]
                b = g % n_groups * GROUP + i
                if not by_bin[b]:
                    return
                ct, ut = st["ct"], st["ut"]
                mg = mg_all[:, st["g"] % n_groups]
                sc = scps.tile([128, 128], F32, tag="sc")
                for k in range(4):
                    nc.tensor.matmul(
                        sc[:], ct[:, k, i, :], ut[:, k, i, :],
                        start=(k == 0), stop=(k == 3))
                nc.vector.tensor_add(sc[:], sc[:], mg[:, i, :])

                nmax = statp.tile([128, 1], F32, tag="nmax")
                sums = statp.tile([128, 1], F32, tag="sums")
                recip = statp.tile([128, 1], F32, tag="recip")
                expt = segp.tile([128, 128], DT, tag="expt")
                nc.vector.tensor_reduce(
                    nmax[:], sc[:], axis=mybir.AxisListType.X,
                    op=mybir.AluOpType.max, negate=True)
                nc.scalar.activation(
                    expt[:], sc[:], mybir.ActivationFunctionType.Exp,
                    bias=nmax[:])
                nc.vector.tensor_reduce(
                    sums[:], expt[:], axis=mybir.AxisListType.X,
                    op=mybir.AluOpType.add)
                nc.vector.reciprocal(recip[:], sums[:])
                st[("bin", i)] = (expt, recip)

            def bin_out(st, i, use_act_copy):
                if ("bin", i) not in st:
                    return
                expt, recip = st.pop(("bin", i))
                cg, og = st["cg"], st["og"]
                tp = teps.tile([128, 128], DT, tag="te")
                nc.tensor.transpose(tp[:], expt[:], id_t[:])
                attn = segp.tile([128, 128], DT, tag="attn")
                nc.vector.tensor_copy(attn[:], tp[:])

                ops_t = opsp.tile([128, D], F32, tag="ops")
                nc.tensor.matmul(ops_t[:], attn[:], cg[:, i, :],
                                 start=True, stop=True)
                # normalize rows by 1/sum during the psum->sbuf copy
                if use_act_copy:
                    nc.scalar.activation(og[:, i, :], ops_t[:],
                                         mybir.ActivationFunctionType.Copy,
                                         scale=recip[:])
                else:
                    nc.vector.tensor_scalar_mul(og[:, i, :], ops_t[:], recip[:])

            def store_group(st):
                g, nb = st["g"] % n_groups, st["nb"]
                # Pool HWDGE queue: keeps the blocking store off the SP load
                # queue and the busy Act/DVE sequencers
                nc.gpsimd.dma_start(
                    opk_v[:, g * GROUP:g * GROUP + nb, :], st["og"][:, :nb, :])

            def do_out(entry):
                st, i = entry
                # 1-of-4 normalize copies on Act, rest on DVE (Act is the
                # second-busiest engine after PE: tanh+exp dominate it)
                bin_out(st, i, use_act_copy=False)
                st["done"] = st.get("done", 0) + 1
                if st["done"] == st["nb"]:
                    store_group(st)

            # 3-deep software pipeline: while group g runs its softmax/out
            # chains on Act/DVE, group g+1's u-matmuls fill the PE stream and
            # group g+2's DMAs are in flight.  bin_outs are deferred through
            # a ~4-deep cross-iteration queue so the in-order PE always has
            # a ready out-matmul to run instead of idling on the
            # exp -> transpose -> attn-copy latency chain.
            niter = repeat * n_groups
            states = {}
            pend = []
            for it in range(niter + 3):
                if it < niter:
                    states[it] = load_group(it % n_groups)
                st_mid = states.get(it - 1)
                st_old = states.pop(it - 2, None)
                if st_old is not None:
                    og_t = outp.tile([128, GROUP, D], DT, tag="og")
                    st_old["og"] = og_t

                for i in range(GROUP):
                    if st_old is not None:
                        if by_bin[st_old["g"] % n_groups * GROUP + i]:
                            bin_scores(st_old, i)
                            pend.append((st_old, i))
                    if st_mid is not None:
                        u_chunk(st_mid, i)
                    while len(pend) > 3:
                        do_out(pend.pop(0))
                if it >= niter:
                    while pend:
                        do_out(pend.pop(0))

    nc.compile()
    return nc


def _host_arrays(slots, n_bins, seg_ids, lengths, context, W, b, mode=None):
    DT = np.float16
    T = n_bins * BIN
    by_bin2 = [[] for _ in range(n_bins)]
    for j, (bi, off, L) in enumerate(slots):
        by_bin2[bi].append((j, off, L))
    n_groups = n_bins // GROUP

    wt = np.ascontiguousarray(
        W.T.reshape(4, 128, D).transpose(1, 0, 2).reshape(128, 4 * D)).astype(DT)
    bvec = np.ascontiguousarray(b.reshape(4, 128).T).astype(np.float32)

    NEG = -30000.0
    in_maps = []
    for c in range(N_CORES):
        cpk = np.zeros((T, D), DT)
        kb = np.full(T, NEG, np.float32)
        for j, (bi, off, _L) in enumerate(slots):
            s = seg_ids[c][j]
            n = int(lengths[s])
            r0 = bi * BIN + off
            cpk[r0:r0 + n] = context[s, :n].astype(DT)
            kb[r0:r0 + n] = 0.0
        # ctt[g*128+p, k*512 + i*128 + t] = cpk[(4g+i)*128 + t, 128k + p]
        ctt = np.ascontiguousarray(
            cpk.reshape(n_groups, GROUP, 128, 4, 128)
               .transpose(0, 4, 3, 1, 2)
               .reshape(n_groups * 128, 4 * GROUP * 128))
        msk = np.zeros((n_groups, 128, GROUP, 128), np.float32)
        for j, (bi, off, _L) in enumerate(slots):
            s = seg_ids[c][j]
            n = int(lengths[s])
            g2, i = divmod(bi, GROUP)
            msk[g2, off:off + n, i, :] = NEG
            msk[g2, off:off + n, i, off:off + n] = 0.0
        import ml_dtypes
        in_maps.append({"cpk": cpk, "ctt": ctt, "wt": wt, "bvec": bvec,
                        "msk": msk.reshape(n_groups * 128, GROUP * 128)
                                  .astype(ml_dtypes.float8_e5m2)})
    return in_maps


_CACHE = {}


def kernel(context, lengths, W, b, mode=None):
    context = np.asarray(context, dtype=np.float32)
    lengths = np.asarray(lengths, dtype=np.int32)
    W = np.asarray(W, dtype=np.float32)
    b = np.asarray(b, dtype=np.float32)
    S, Lmax, Din = context.shape

    slots, n_bins, seg_ids = _plan(lengths)
    key = (tuple(slots), n_bins)
    if key in _CACHE:
        nc = _CACHE[key]
    else:
        nc = _build(slots, n_bins)
        _CACHE[key] = nc

    in_maps = _host_arrays(slots, n_bins, seg_ids, lengths, context, W, b)
    res = run_bass_kernel_spmd(nc, in_maps, list(range(N_CORES)))
    LAST_RESULTS["exec_time_ns"] = res.exec_time_ns

    out = np.zeros((S, Lmax, D), np.float32)
    for c in range(N_CORES):
        opk = res.results[c]["opk"].astype(np.float32)
        for j, (bi, off, _L) in enumerate(slots):
            s = seg_ids[c][j]
            n = int(lengths[s])
            r0 = bi * BIN + off
            out[s, :n] = opk[r0:r0 + n]
    return out


# revision 32
# speedup vs baseline: 1.8034x; 1.8034x over previous
"""Ragged-segment attention for Trainium2 (8 NeuronCores, SPMD), bin-dense fp16.

Per-segment masking/softmax structure is folded into a host-built low-rank
additive mask applied with ONE matmul per bin:
    mask[q,k] = (kb[k] + NEG) * 1  +  sum_s (-NEG) * 1_s[q] 1_s[k]
so scores/softmax/exp-transpose/out are all dense [128 x 128] bin ops and
segments pack at arbitrary offsets (first-fit decreasing, ~94% dense bins).

The feature-major context copy (ctT) is pre-transposed on the HOST and DMA'd
directly, so the PE runs a pure matmul stream (no transpose->PSUM->copy
chains): per 4-bin group 16 u-matmuls, 4x(4 score + 1 mask) matmuls, 4 exp
transposes and 4 out matmuls = 13312 PE cycles.  The 3-deep software
pipeline (load g+2 / u-matmul g+1 / softmax+out g) keeps the in-order PE
from idling while Act/DVE run the softmax stats.

DMAs are batched per 4-bin group (context both layouts, masks, outputs)
because each DMA instruction costs ~625ns of serialized HWDGE
descriptor-generation time.
"""
import numpy as np

import concourse.bacc as bacc
import concourse.mybir as mybir
import concourse.tile as tile
from concourse.bass_utils import run_bass_kernel_spmd

F32 = mybir.dt.float32
FP16 = mybir.dt.float16

N_CORES = 8
D = 512
BIN = 128
GROUP = 4

DEFAULT_MODE = "fp16T"

LAST_RESULTS = {}


def _plan(lengths, mode=None):
    S = len(lengths)
    n_slots = S // N_CORES
    order = np.argsort(-lengths, kind="stable")
    seg_ids = [[int(order[N_CORES * j + c]) for j in range(n_slots)]
               for c in range(N_CORES)]
    slot_len = [int(lengths[order[N_CORES * j]]) for j in range(n_slots)]

    bins = []   # (used-token count, n_segs) per bin
    slots = []  # (bin, off, L)
    for j, L in enumerate(slot_len):
        bi = next((i for i, (used, ns) in enumerate(bins)
                   if used + L <= BIN and ns < 31), None)
        if bi is None:
            bins.append((0, 0))
            bi = len(bins) - 1
        used, ns = bins[bi]
        slots.append((bi, used, L))
        bins[bi] = (used + L, ns + 1)
    n_bins = ((len(bins) + GROUP - 1) // GROUP) * GROUP
    return slots, n_bins, seg_ids


def _mask_layout(slots, n_bins):
    by_bin = [[] for _ in range(n_bins)]
    for bi, off, L in slots:
        by_bin[bi].append((off, L))
    kmask = [len(by_bin[b]) + 1 for b in range(n_bins)]
    assert max(kmask) <= 32
    return by_bin, kmask


def _build(slots, n_bins, mode=None, repeat=1):
    DT = FP16
    T = n_bins * BIN
    n_groups = n_bins // GROUP
    nc = bacc.Bacc("TRN2", target_bir_lowering=False)

    by_bin, kmask = _mask_layout(slots, n_bins)

    cpk = nc.dram_tensor("cpk", [T, D], DT, kind="ExternalInput")
    # feature-major context, host-transposed: row g*128+p holds, for
    # d-partition p of group g, all [k-chunk][bin][token] values
    ctt = nc.dram_tensor("ctt", [n_groups * 128, 4 * GROUP * 128], DT,
                         kind="ExternalInput")
    wt = nc.dram_tensor("wt", [128, 4 * D], DT, kind="ExternalInput")
    bvec = nc.dram_tensor("bvec", [128, 4], F32, kind="ExternalInput")
    # per-group mask rows: bin i of a group at partitions [32i, 32i+km)
    msk = nc.dram_tensor("msk", [n_groups * 128, GROUP * 128], mybir.dt.float8e5,
                         kind="ExternalInput")
    opk = nc.dram_tensor("opk", [T, D], DT, kind="ExternalOutput")

    ident = nc.inline_tensor(np.eye(128, dtype=np.float16), name="ident")

    with tile.TileContext(nc) as tc:
        with (
            tc.tile_pool(name="const", bufs=1) as cpool,
            tc.tile_pool(name="cb", bufs=4) as cbp,
            tc.tile_pool(name="ctp", bufs=3) as ctp,
            tc.tile_pool(name="utp", bufs=2) as utp,
            tc.tile_pool(name="seg", bufs=10) as segp,
            tc.tile_pool(name="stat", bufs=18) as statp,
            tc.tile_pool(name="outp", bufs=3) as outp,
            tc.tile_pool(name="mk", bufs=3) as mkp,
            tc.tile_pool(name="ups", bufs=2, space="PSUM") as ups,
            tc.tile_pool(name="scps", bufs=2, space="PSUM") as scps,
            tc.tile_pool(name="teps", bufs=2, space="PSUM") as teps,
            tc.tile_pool(name="ops", bufs=2, space="PSUM") as opsp,
        ):
            wt_sb = cpool.tile([128, 4, D], DT, tag="wt")
            b_sb = cpool.tile([128, 4], F32, tag="b")
            id_t = cpool.tile([128, 128], DT, tag="id")
            mg_all = cpool.tile([128, n_groups, GROUP, 128], mybir.dt.float8e5,
                                tag="mg_all")
            cg_all = cpool.tile([128, n_groups, GROUP, D], DT, tag="cg_all")
            ct_all = cpool.tile([128, n_groups, 4, GROUP, 128], DT, tag="ct_all")
            nc.sync.dma_start(wt_sb[:], wt.ap().rearrange("p (c e) -> p c e", c=4))
            nc.sync.dma_start(b_sb[:], bvec[:])
            nc.sync.dma_start(id_t[:], ident[:])
            # all groups' mask rows stay SBUF-resident (4.25KB/partition):
            # saves a 32KB DMA per group of steady-state HBM traffic
            nc.sync.dma_start(
                mg_all[:], msk.ap().rearrange("(g r) (i p) -> r g i p",
                                              i=GROUP, g=n_groups))
            # the WHOLE packed context stays SBUF-resident in both layouts
            # (68KB/partition each): steady-state HBM traffic is then just
            # the output store
            cpk_r = cpk.ap().rearrange("(g i p) d -> p g i d", p=BIN, i=GROUP)
            ctt_r = ctt.ap().rearrange("(g p) (k i t) -> p g k i t",
                                       p=128, k=4, i=GROUP)
            for gq in range(0, n_groups, 5):
                ge = min(gq + 5, n_groups)
                nc.sync.dma_start(cg_all[:, gq:ge], cpk_r[:, gq:ge])
                nc.gpsimd.dma_start(ct_all[:, gq:ge], ctt_r[:, gq:ge])

            cpk_v = cpk.ap().rearrange("(b p) d -> p b d", p=BIN)
            opk_v = opk.ap().rearrange("(b p) d -> p b d", p=BIN)
            ctt_v = ctt.ap().rearrange("(g p) (k i t) -> g p k i t",
                                       p=128, k=4, i=GROUP)

            # non-empty bins are a prefix of each group (packing fills bins
            # in order), so per-group work/stores cover just the first nb
            nb_used = [sum(1 for i in range(GROUP) if by_bin[g * GROUP + i])
                       for g in range(n_groups)]

            def load_group(g):
                """Context is resident: just bind the group's slices."""
                return {"g": g, "nb": nb_used[g],
                        "cg": cg_all[:, g], "ct": ct_all[:, g]}

            def u_chunk(st, c):
                ct, nb = st["ct"], st["nb"]
                if c == 0:
                    ut_t = utp.tile([128, 4, GROUP, 128], DT, tag="ut")
                    st["ut"] = ut_t
                ups_t = ups.tile([128, GROUP * 128], F32, tag="ups")
                for k in range(4):
                    nc.tensor.matmul(
                        ups_t[:, :nb * 128], wt_sb[:, k, c * 128:(c + 1) * 128],
                        ct[:, k, :nb, :], start=(k == 0), stop=(k == 3))
                nc.scalar.activation(
                    st["ut"][:, c, :nb, :], ups_t[:, :nb * 128],
                    mybir.ActivationFunctionType.Tanh, bias=b_sb[:, c:c + 1])

            def bin_scores(st, i):
                g = st["g"]
                b = g % n_groups * GROUP + i
                if not by_bin[b]:
                    return
                ct, ut = st["ct"], st["ut"]
                mg = mg_all[:, st["g"] % n_groups]
                sc = scps.tile([128, 128], F32, tag="sc")
                for k in range(4):
                    nc.tensor.matmul(
                        sc[:], ct[:, k, i, :], ut[:, k, i, :],
                        start=(k == 0), stop=(k == 3))
                nc.vector.tensor_add(sc[:], sc[:], mg[:, i, :])

                nmax = statp.tile([128, 1], F32, tag="nmax")
                sums = statp.tile([128, 1], F32, tag="sums")
                recip = statp.tile([128, 1], F32, tag="recip")
                expt = segp.tile([128, 128], DT, tag="expt")
                nc.vector.tensor_reduce(
                    nmax[:], sc[:], axis=mybir.AxisListType.X,
                    op=mybir.AluOpType.max, negate=True)
                nc.scalar.activation(
                    expt[:], sc[:], mybir.ActivationFunctionType.Exp,
                    bias=nmax[:], accum_out=sums[:])
                nc.vector.reciprocal(recip[:], sums[:])
                st[("bin", i)] = (expt, recip)

            def bin_out(st, i, use_act_copy):
                if ("bin", i) not in st:
                    return
                expt, recip = st.pop(("bin", i))
                cg, og = st["cg"], st["og"]
                tp = teps.tile([128, 128], DT, tag="te")
                nc.tensor.transpose(tp[:], expt[:], id_t[:])
                attn = segp.tile([128, 128], DT, tag="attn")
                nc.vector.tensor_copy(attn[:], tp[:])

                ops_t = opsp.tile([128, D], F32, tag="ops")
                nc.tensor.matmul(ops_t[:], attn[:], cg[:, i, :],
                                 start=True, stop=True)
                # normalize rows by 1/sum during the psum->sbuf copy
                if use_act_copy:
                    nc.scalar.activation(og[:, i, :], ops_t[:],
                                         mybir.ActivationFunctionType.Copy,
                                         scale=recip[:])
                else:
                    nc.vector.tensor_scalar_mul(og[:, i, :], ops_t[:], recip[:])

            def store_group(st):
                g, nb = st["g"] % n_groups, st["nb"]
                # Pool HWDGE queue: keeps the blocking store off the SP load
                # queue and the busy Act/DVE sequencers
                nc.gpsimd.dma_start(
                    opk_v[:, g * GROUP:g * GROUP + nb, :], st["og"][:, :nb, :])

            def do_out(entry):
                st, i = entry
                # 1-of-4 normalize copies on Act, rest on DVE (Act is the
                # second-busiest engine after PE: tanh+exp dominate it)
                bin_out(st, i, use_act_copy=(i == 0))
                st["done"] = st.get("done", 0) + 1
                if st["done"] == st["nb"]:
                    store_group(st)

            # 3-deep software pipeline: while group g runs its softmax/out
            # chains on Act/DVE, group g+1's u-matmuls fill the PE stream and
            # group g+2's DMAs are in flight.  bin_outs are deferred through
            # a ~4-deep cross-iteration queue so the in-order PE always has
            # a ready out-matmul to run instead of idling on the
            # exp -> transpose -> attn-copy latency chain.
            niter = repeat * n_groups
            states = {}
            pend = []
            for it in range(niter + 3):
                if it < niter:
                    states[it] = load_group(it % n_groups)
                st_mid = states.get(it - 1)
                st_old = states.pop(it - 2, None)
                if st_old is not None:
                    og_t = outp.tile([128, GROUP, D], DT, tag="og")
                    st_old["og"] = og_t

                for i in range(GROUP):
                    if st_old is not None:
                        if by_bin[st_old["g"] % n_groups * GROUP + i]:
                            bin_scores(st_old, i)
                            pend.append((st_old, i))
                    if st_mid is not None:
                        u_chunk(st_mid, i)
                    while len(pend) > 3:
                        do_out(pend.pop(0))
                if it >= niter:
                    while pend:
                        do_out(pend.pop(0))

    nc.compile()
    return nc


def _host_arrays(slots, n_bins, seg_ids, lengths, context, W, b, mode=None):
    DT = np.float16
    T = n_bins * BIN
    by_bin2 = [[] for _ in range(n_bins)]
    for j, (bi, off, L) in enumerate(slots):
        by_bin2[bi].append((j, off, L))
    n_groups = n_bins // GROUP

    wt = np.ascontiguousarray(
        W.T.reshape(4, 128, D).transpose(1, 0, 2).reshape(128, 4 * D)).astype(DT)
    bvec = np.ascontiguousarray(b.reshape(4, 128).T).astype(np.float32)

    NEG = -30000.0
    in_maps = []
    for c in range(N_CORES):
        cpk = np.zeros((T, D), DT)
        kb = np.full(T, NEG, np.float32)
        for j, (bi, off, _L) in enumerate(slots):
            s = seg_ids[c][j]
            n = int(lengths[s])
            r0 = bi * BIN + off
            cpk[r0:r0 + n] = context[s, :n].astype(DT)
            kb[r0:r0 + n] = 0.0
        # ctt[g*128+p, k*512 + i*128 + t] = cpk[(4g+i)*128 + t, 128k + p]
        ctt = np.ascontiguousarray(
            cpk.reshape(n_groups, GROUP, 128, 4, 128)
               .transpose(0, 4, 3, 1, 2)
               .reshape(n_groups * 128, 4 * GROUP * 128))
        msk = np.zeros((n_groups, 128, GROUP, 128), np.float32)
        for j, (bi, off, _L) in enumerate(slots):
            s = seg_ids[c][j]
            n = int(lengths[s])
            g2, i = divmod(bi, GROUP)
            msk[g2, off:off + n, i, :] = NEG
            msk[g2, off:off + n, i, off:off + n] = 0.0
        import ml_dtypes
        in_maps.append({"cpk": cpk, "ctt": ctt, "wt": wt, "bvec": bvec,
                        "msk": msk.reshape(n_groups * 128, GROUP * 128)
                                  .astype(ml_dtypes.float8_e5m2)})
    return in_maps


_CACHE = {}


def kernel(context, lengths, W, b, mode=None):
    context = np.asarray(context, dtype=np.float32)
    lengths = np.asarray(lengths, dtype=np.int32)
    W = np.asarray(W, dtype=np.float32)
    b = np.asarray(b, dtype=np.float32)
    S, Lmax, Din = context.shape

    slots, n_bins, seg_ids = _plan(lengths)
    key = (tuple(slots), n_bins)
    if key in _CACHE:
        nc = _CACHE[key]
    else:
        nc = _build(slots, n_bins)
        _CACHE[key] = nc

    in_maps = _host_arrays(slots, n_bins, seg_ids, lengths, context, W, b)
    res = run_bass_kernel_spmd(nc, in_maps, list(range(N_CORES)))
    LAST_RESULTS["exec_time_ns"] = res.exec_time_ns

    out = np.zeros((S, Lmax, D), np.float32)
    for c in range(N_CORES):
        opk = res.results[c]["opk"].astype(np.float32)
        for j, (bi, off, _L) in enumerate(slots):
            s = seg_ids[c][j]
            n = int(lengths[s])
            r0 = bi * BIN + off
            out[s, :n] = opk[r0:r0 + n]
    return out
